# revision 32
# baseline (speedup 1.0000x reference)
# BitAttention (ternary-quantized GQA transformer block) on 8 Trainium2 NeuronCores.
#
# Reference computation (see problem):
#   w_q = sign(w) * mean(|w|)            (per weight tensor, global scale)
#   q = x @ w_q(wq).T ; k = x @ w_q(wk).T ; v = x @ w_q(wv).T
#   GQA causal attention (32 q heads, 8 kv heads, head_dim 64)
#   out = attn @ w_q(wo).T
#
# Sharding (8 cores): batch (2) x kv-head-group (4).  Each core computes
# attention for 2 kv heads / 8 q heads of one batch and a partial out-proj
# over its 512 attention-output features; the host sums 4 partials per batch.
#
# Device layout: activations are kept feature-major ("transposed", [feat, token]).
# Inputs enter pre-transposed/sliced in bf16 (the kernel's compute precision);
# sign() is computed on device; the global quant scales enter as a tiny [1,2]
# fp32 tensor and are folded into the softmax exp() scale (sq*sk/sqrt(hd)) and
# the V-transpose copy (sv*so).  All matmuls are bf16 with fp32 PSUM
# accumulation; the returned partials are summed in fp32 on the host.
#
# Softmax runs without max-subtraction (scores are O(1) by construction:
# mean|w|^2-scaled dot products), so exp/rowsum/divide is exact enough in
# fp32 PSUM + bf16 storage.  Scores are computed transposed ([key, query])
# so the PV matmul needs no transposes; the rowsum rides along as a "ones"
# column appended to V; 1/rowsum is exp(-ln r) on ACT (one table set, loaded
# once).  The two heads of a pair sit on different PE row strips so their
# score matmuls execute concurrently; PV lags scores by one chunk and V-
# transposes / out-proj groups are dripped between chunks to keep the PE
# busy (HAM clock-gate warm).

import sys

for _p in ("/opt/trn_rl_repo",):
    if _p not in sys.path:
        sys.path.append(_p)

import numpy as np
import ml_dtypes

import concourse.bass as bass
import concourse.tile as tile
from concourse import bacc, mybir
from concourse import bass_utils
from concourse.masks import make_identity

F32 = mybir.dt.float32
BF16 = mybir.dt.bfloat16
ALU = mybir.AluOpType
ACT = mybir.ActivationFunctionType

D = 2048          # model dim
S = 2048          # sequence length
B = 2             # batch
HD = 64           # head dim
NQH = 8           # q heads per core
NKV = 2           # kv heads per core
QF = NQH * HD     # 512 q features per core
KF = NKV * HD     # 128 kv features per core
QB = 512          # query block (free dim of score matmuls)
KT = 128          # key tile (partition dim of transposed scores)
NKT = S // KT     # 16
NQB = S // QB     # 4
NDT = D // 128    # 16 contraction tiles
EPS = 1e-5

# processing order of local q heads: tile ft holds heads (ft, ft+4) so that
# the head's row block (64*(h//4)) matches its kv head's row block in k_sb.
PERM = [0, 4, 1, 5, 2, 6, 3, 7]

_NC = None
_LAST_RESULTS = None


def _build():
    nc = bacc.Bacc("TRN2", target_bir_lowering=False, debug=False, num_devices=8)

    xt_d = nc.dram_tensor("xt", [D, S], BF16, kind="ExternalInput")
    wqt_d = nc.dram_tensor("wqt", [D, QF], BF16, kind="ExternalInput")
    wkt_d = nc.dram_tensor("wkt", [D, KF], BF16, kind="ExternalInput")
    wvt_d = nc.dram_tensor("wvt", [D, KF], BF16, kind="ExternalInput")
    wot_d = nc.dram_tensor("wot", [QF, D], BF16, kind="ExternalInput")
    sc_d = nc.dram_tensor("sc", [1, 2], F32, kind="ExternalInput")
    yt_d = nc.dram_tensor("yt", [D, S], BF16, kind="ExternalOutput")

    with tile.TileContext(nc) as tc:
        with (
            tc.tile_pool(name="persist", bufs=1) as pers,
            tc.tile_pool(name="stg", bufs=2) as stg,
            tc.tile_pool(name="work", bufs=2) as work,
            tc.tile_pool(name="exps_p", bufs=4) as exps_p,
            tc.tile_pool(name="ysb_p", bufs=3) as ysb_p,
            tc.tile_pool(name="mm", bufs=2, space="PSUM") as mm,
            tc.tile_pool(name="scp", bufs=2, space="PSUM") as scp,
            tc.tile_pool(name="pop", bufs=2, space="PSUM") as pop,
        ):
            # ---- constants ----
            sscore_bc = pers.tile([128, 1], F32, tag="sscore")
            sout_bc = pers.tile([128, 1], F32, tag="sout")
            nc.sync.dma_start(out=sscore_bc, in_=sc_d[0:1, 0:1].to_broadcast([128, 1]))
            nc.sync.dma_start(out=sout_bc, in_=sc_d[0:1, 1:2].to_broadcast([128, 1]))
            ident = pers.tile([128, 128], BF16, tag="ident")
            make_identity(nc, ident)
            # ones row at partition 64 for the rowsum-broadcast matmul
            ones64 = pers.tile([HD + 1, HD], F32, tag="ones64")
            nc.gpsimd.memset(ones64, 1.0)

            # ---- load + sign-quantize weights (device-side sign -> bf16) ----
            def load_sign(dram, cols, tile_range, tagbase):
                # stage/sign in <=512-column chunks to keep staging slots small
                cw = min(cols, 512)
                out_tiles = []
                for t in tile_range:
                    wsb = pers.tile([128, cols], BF16, tag=f"{tagbase}{t}", name=f"{tagbase}{t}")
                    for c0 in range(0, cols, cw):
                        wstg = stg.tile([128, cw], BF16, tag="wstg")
                        nc.sync.dma_start(
                            out=wstg, in_=dram[t * 128:(t + 1) * 128, c0:c0 + cw]
                        )
                        wtmp = stg.tile([128, cw], BF16, tag="wtmp")
                        # (w >= 0) * 2 -> {2, 0}
                        nc.vector.tensor_scalar(wtmp, wstg, 0.0, 2.0, ALU.is_ge, ALU.mult)
                        # {2,0} - 1 -> {1,-1}
                        nc.vector.tensor_scalar(wsb[:, c0:c0 + cw], wtmp, 1.0, None, ALU.subtract)
                    out_tiles.append(wsb)
                return out_tiles

            # ---- load + quantize, interleaved by contraction tile so the
            # projections (which consume tile t of x and w together) can start
            # as soon as the early tiles land, hiding the load under PE work.
            # wo is loaded last: it is first needed an entire query-block later.
            x_sb = []
            wk_sb, wv_sb, wq_sb = [], [], []
            for t in range(NDT):
                xsb = pers.tile([128, S], BF16, tag=f"x{t}", name=f"x{t}")
                nc.sync.dma_start(out=xsb, in_=xt_d[t * 128:(t + 1) * 128, :])
                x_sb.append(xsb)
                wk_sb += load_sign(wkt_d, KF, range(t, t + 1), "wk")
                wv_sb += load_sign(wvt_d, KF, range(t, t + 1), "wv")
                wq_sb += load_sign(wqt_d, QF, range(t, t + 1), "wq")

            # ---- projections (feature-major: out[feat, token]) ----
            def project(w_tiles, w_col0, out_sb, out_col0):
                # out_sb[:, qb block] = (sum_kt w[kt][:, cols].T @ x[kt][:, qb]) as bf16
                for qb in range(NQB):
                    ps = mm.tile([128, QB], F32, tag="mm")
                    for t in range(NDT):
                        nc.tensor.matmul(
                            ps,
                            w_tiles[t][:, w_col0:w_col0 + 128],
                            x_sb[t][:, qb * QB:(qb + 1) * QB],
                            start=(t == 0),
                            stop=(t == NDT - 1),
                        )
                    nc.vector.tensor_copy(out_sb[:, out_col0 + qb * QB:out_col0 + (qb + 1) * QB], ps)

            k_sb = pers.tile([128, S], BF16, tag="ksb")
            project(wk_sb, 0, k_sb, 0)

            vf_sb = pers.tile([128, S], BF16, tag="vfsb")
            project(wv_sb, 0, vf_sb, 0)

            # causal masks for the 4 diagonal key-tile offsets:
            # mask[d][p, f] = 1.0 where f >= p + 128*d else 0.0
            dmask = []
            for dmi in range(4):
                msk = pers.tile([128, QB], BF16, tag=f"dmask{dmi}", name=f"dmask{dmi}")
                nc.gpsimd.memset(msk, 1.0)
                nc.gpsimd.affine_select(
                    out=msk, in_=msk, compare_op=ALU.is_ge, fill=0.0,
                    base=-128 * dmi, pattern=[[1, QB]], channel_multiplier=-1,
                )
                dmask.append(msk)

            # token-major V with a trailing ones column:
            # vtok[t][:, kv, 0:64] = V.T * (sv*so).  Only the first 4 key tiles
            # are produced up front; the rest are dripped into the attention
            # loop as PE filler (they are not needed until later query blocks).
            vtok = [
                pers.tile([128, NKV, HD + 1], BF16, tag=f"vtok{t}", name=f"vtok{t}")
                for t in range(NKT)
            ]

            def emit_vtok(t):
                vt = vtok[t]
                pst = mm.tile([128, 128], BF16, tag="mm")
                nc.tensor.transpose(pst, vf_sb[:, t * 128:(t + 1) * 128], ident)
                for kv in range(NKV):
                    nc.vector.tensor_scalar(
                        vt[:, kv, 0:HD], pst[:, kv * HD:(kv + 1) * HD],
                        sout_bc, None, ALU.mult,
                    )
                nc.vector.memset(vt[:, :, HD:HD + 1], 1.0)

            for t in range(4):
                emit_vtok(t)

            o_sb = [
                pers.tile([128, S], BF16, tag=f"osb{ft}", name=f"osb{ft}")
                for ft in range(4)
            ]
            q_sb = [
                pers.tile([128, S], BF16, tag=f"qsb{ft}", name=f"qsb{ft}")
                for ft in range(4)
            ]
            wo_sb = None  # loaded lazily after the first Q projection

            def emit_ygroup(qb, ot):
                # one partial out-projection psum group for query block qb
                q0 = qb * QB
                py = mm.tile([128, QB], F32, tag="mm")
                for it in range(4):
                    nc.tensor.matmul(
                        py,
                        wo_sb[it][:, ot * 128:(ot + 1) * 128],
                        o_sb[it][:, q0:q0 + QB],
                        start=(it == 0),
                        stop=(it == 3),
                    )
                ysb = ysb_p.tile([128, QB], BF16, tag="ysb")
                nc.vector.tensor_copy(ysb, py)
                nc.sync.dma_start(out=yt_d[ot * 128:(ot + 1) * 128, q0:q0 + QB], in_=ysb)

            # PE filler queue: small dense PE tasks (V transposes, Y-proj
            # groups for completed query blocks) dripped one per attention
            # chunk so the PE never idles while ACT computes exps (keeps the
            # HAM clock-gate warm).
            filler = [(emit_vtok, (t,)) for t in range(4, NKT)]

            def drip():
                if filler:
                    fn, args = filler.pop(0)
                    fn(*args)

            # attention: per (query block, q-tile): process the head pair
            # (ft -> rows 0:64, ft+4 -> rows 64:128) with score matmuls for the
            # two heads adjacent (they run concurrently on different PE row
            # strips) and PV lagging scores by one chunk so PE never waits on
            # the ACT exp.
            for qb in range(NQB):
                q0 = qb * QB
                nkt = 4 * (qb + 1)          # causal: key tiles 0..nkt-1
                nch = nkt // 2              # chunks of 2 key tiles
                if qb > 0:
                    filler.extend(
                        (emit_ygroup, (qb - 1, ot)) for ot in range(NDT)
                    )
                for ft in range(4):
                    if qb == 0:
                        # produce Q for this q-tile just-in-time; the next
                        # tile's projection then fills PE while this tile's
                        # attention waits on ACT.
                        project(wq_sb, ft * 128, q_sb[ft], 0)
                        if ft == 0:
                            wo_sb = load_sign(wot_d, D, range(QF // 128), "wo")
                    po_ = [
                        pop.tile([HD + 1, QB], F32, tag="po", name=f"po{qb}_{ft}_{p}")
                        for p in range(2)
                    ]

                    def emit_pv(kt, ex):
                        for p in range(2):
                            nc.tensor.matmul(
                                po_[p],
                                vtok[kt][:, p, :],
                                ex[:, p, :],
                                start=(kt == 0),
                                stop=(kt == nkt - 1),
                            )

                    prev = None
                    for kt in range(nkt):
                        # both heads' scores for one key tile in a 2-bank psum
                        # tile; bufs=2 so the next tile's scores run on PE
                        # while ACT computes this tile's exp.
                        ps = scp.tile([128, 2, QB], F32, tag="sc", bufs=2,
                                      name=f"sc{qb}_{ft}_{kt}")
                        k0 = kt * KT
                        for p in range(2):
                            r0 = p * HD
                            nc.tensor.matmul(
                                ps[:, p, :],
                                k_sb[r0:r0 + HD, k0:k0 + KT],
                                q_sb[ft][r0:r0 + HD, q0:q0 + QB],
                                start=True, stop=True,
                            )
                        ex = exps_p.tile([128, 2, QB], BF16, tag="ex", bufs=3,
                                         name=f"ex{qb}_{ft}_{kt}")
                        nc.scalar.activation(
                            out=ex[:, :, :], in_=ps[:, :, :],
                            func=ACT.Exp, scale=sscore_bc,
                        )
                        if kt >= 4 * qb:  # diagonal tile: apply causal mask
                            dmi = kt - 4 * qb
                            for p in range(2):
                                nc.vector.tensor_tensor(
                                    ex[:, p, :], ex[:, p, :], dmask[dmi], ALU.mult,
                                )
                        if prev is not None:
                            emit_pv(kt - 1, prev)
                        prev = ex
                        if kt % 2 == 0:
                            drip()
                    emit_pv(nkt - 1, prev)

                    # normalize: O[:, q] * (1 / rowsum[q]); rowsum is po row 64.
                    # Entirely off ACT (it paces the late phase): copy the
                    # rowsum row to SBUF on DVE, broadcast it across 64
                    # partitions with a K=1 ones-matmul, then take the
                    # reciprocal on DVE (approx_fast, ~4e-6 rel; inputs are
                    # well-conditioned rowsums >= 1ulp of exp(0)).
                    for p in range(2):
                        # copy both po regions out first so the PSUM slot frees
                        # early (the next pair's PV matmuls reuse it); the rest
                        # of the chain runs off SBUF.
                        otmp = work.tile([HD, QB], BF16, tag="otmp")
                        nc.vector.tensor_copy(otmp, po_[p][0:HD, :])
                        rsum = work.tile([HD + 1, QB], F32, tag="rsum")
                        nc.vector.tensor_copy(rsum[HD:HD + 1, :], po_[p][HD:HD + 1, :])
                        bcp = mm.tile([HD, QB], F32, tag="mm")
                        nc.tensor.matmul(
                            bcp,
                            ones64[HD:HD + 1, :],
                            rsum[HD:HD + 1, :],
                            start=True, stop=True,
                        )
                        bcr = work.tile([HD, QB], F32, tag="bcr")
                        nc.vector.reciprocal_approx_fast(out=bcr, in_=bcp)
                        if p == 0:
                            # head 0 lands on partitions 0-63: write in place
                            nc.vector.tensor_tensor(
                                o_sb[ft][0:HD, q0:q0 + QB], otmp, bcr, ALU.mult
                            )
                        else:
                            ostg = work.tile([HD, QB], BF16, tag="ostg")
                            nc.vector.tensor_tensor(ostg, otmp, bcr, ALU.mult)
                            nc.sync.dma_start(
                                out=o_sb[ft][HD:2 * HD, q0:q0 + QB], in_=ostg
                            )

            # drain remaining filler (spilled Y groups) and the last block
            while filler:
                drip()
            for ot in range(NDT):
                emit_ygroup(NQB - 1, ot)

    # The ACT table-set selector assigns Exp -> exp_and_others and
    # Ln -> natural_log (first set containing each func), which thrashes the
    # table RAM (~2.7us per switch) on every ln<->exp transition in the
    # normalization chain.  Both live in natural_log_exp_and_others; steer the
    # selector there by hiding exp/ln from the other sets during this compile.
    import concourse.bacc as bacc_mod

    orig_tables = bacc_mod.get_activation_tables

    def one_set_tables(arch):
        t = orig_tables(arch)
        for name, fns in t.items():
            if name != "natural_log_exp_and_others":
                fns.discard(ACT.Exp)
                fns.discard(ACT.Ln)
        return t

    bacc_mod.get_activation_tables = one_set_tables
    try:
        nc.compile()
    finally:
        bacc_mod.get_activation_tables = orig_tables
    return nc


def _get_nc():
    global _NC
    if _NC is None:
        _NC = _build()
    return _NC


def run(inputs, trace=False, trace_cores=None):
    global _LAST_RESULTS
    x = np.asarray(inputs["x"], dtype=np.float32)
    wq = np.asarray(inputs["wq"], dtype=np.float32)
    wk = np.asarray(inputs["wk"], dtype=np.float32)
    wv = np.asarray(inputs["wv"], dtype=np.float32)
    wo = np.asarray(inputs["wo"], dtype=np.float32)

    sq = max(np.abs(wq).mean(), EPS)
    sk = max(np.abs(wk).mean(), EPS)
    sv = max(np.abs(wv).mean(), EPS)
    so = max(np.abs(wo).mean(), EPS)
    sc = np.array([[sq * sk / np.sqrt(HD), sv * so]], dtype=np.float32)

    perm_rows = np.concatenate([np.arange(h * HD, (h + 1) * HD) for h in PERM])

    in_maps = []
    for c in range(8):
        b, g = divmod(c, 4)
        wq_g = wq[QF * g:QF * (g + 1), :][perm_rows]        # [512, 2048]
        wk_g = wk[KF * g:KF * (g + 1), :]                   # [128, 2048]
        wv_g = wv[KF * g:KF * (g + 1), :]
        wo_g = wo[:, QF * g:QF * (g + 1)][:, perm_rows]     # [2048, 512]
        bf = ml_dtypes.bfloat16
        in_maps.append({
            "xt": np.ascontiguousarray(x[b].T).astype(bf),
            "wqt": np.ascontiguousarray(wq_g.T).astype(bf),
            "wkt": np.ascontiguousarray(wk_g.T).astype(bf),
            "wvt": np.ascontiguousarray(wv_g.T).astype(bf),
            "wot": np.ascontiguousarray(wo_g.T).astype(bf),
            "sc": sc,
        })

    nc = _get_nc()
    kwargs = {}
    if trace:
        kwargs["trace"] = True
        kwargs["trace_cores"] = trace_cores if trace_cores is not None else [0]
    res = bass_utils.run_bass_kernel_spmd(nc, in_maps, list(range(8)), **kwargs)
    _LAST_RESULTS = res

    y = np.empty((B, S, D), dtype=np.float32)
    for b in range(B):
        acc = np.zeros((D, S), dtype=np.float32)
        for g in range(4):
            acc += res.results[4 * b + g]["yt"].astype(np.float32)
        y[b] = acc.T
    return y


def kernel(**inputs):
    return run(inputs, trace=False)


# revision 33
# speedup vs baseline: 1.0122x; 1.0122x over previous
# BitAttention (ternary-quantized GQA transformer block) on 8 Trainium2 NeuronCores.
#
# Reference computation (see problem):
#   w_q = sign(w) * mean(|w|)            (per weight tensor, global scale)
#   q = x @ w_q(wq).T ; k = x @ w_q(wk).T ; v = x @ w_q(wv).T
#   GQA causal attention (32 q heads, 8 kv heads, head_dim 64)
#   out = attn @ w_q(wo).T
#
# Sharding (8 cores): batch (2) x kv-head-group (4).  Each core computes
# attention for 2 kv heads / 8 q heads of one batch and a partial out-proj
# over its 512 attention-output features; the host sums 4 partials per batch.
#
# Device layout: activations are kept feature-major ("transposed", [feat, token]).
# Inputs enter pre-transposed/sliced in bf16 (the kernel's compute precision);
# sign() is computed on device; the global quant scales enter as a tiny [1,2]
# fp32 tensor and are folded into the softmax exp() scale (sq*sk/sqrt(hd)) and
# the V-transpose copy (sv*so).  All matmuls are bf16 with fp32 PSUM
# accumulation; the returned partials are summed in fp32 on the host.
#
# Softmax runs without max-subtraction (scores are O(1) by construction:
# mean|w|^2-scaled dot products), so exp/rowsum/divide is exact enough in
# fp32 PSUM + bf16 storage.  Scores are computed transposed ([key, query])
# so the PV matmul needs no transposes; the rowsum rides along as a "ones"
# column appended to V; 1/rowsum is exp(-ln r) on ACT (one table set, loaded
# once).  The two heads of a pair sit on different PE row strips so their
# score matmuls execute concurrently; PV lags scores by one chunk and V-
# transposes / out-proj groups are dripped between chunks to keep the PE
# busy (HAM clock-gate warm).

import sys

for _p in ("/opt/trn_rl_repo",):
    if _p not in sys.path:
        sys.path.append(_p)

import numpy as np
import ml_dtypes

import concourse.bass as bass
import concourse.tile as tile
from concourse import bacc, mybir
from concourse import bass_utils
from concourse.masks import make_identity

F32 = mybir.dt.float32
BF16 = mybir.dt.bfloat16
ALU = mybir.AluOpType
ACT = mybir.ActivationFunctionType

D = 2048          # model dim
S = 2048          # sequence length
B = 2             # batch
HD = 64           # head dim
NQH = 8           # q heads per core
NKV = 2           # kv heads per core
QF = NQH * HD     # 512 q features per core
KF = NKV * HD     # 128 kv features per core
QB = 512          # query block (free dim of score matmuls)
KT = 128          # key tile (partition dim of transposed scores)
NKT = S // KT     # 16
NQB = S // QB     # 4
NDT = D // 128    # 16 contraction tiles
EPS = 1e-5

# processing order of local q heads: tile ft holds heads (ft, ft+4) so that
# the head's row block (64*(h//4)) matches its kv head's row block in k_sb.
PERM = [0, 4, 1, 5, 2, 6, 3, 7]

_NC = None
_LAST_RESULTS = None


def _build():
    nc = bacc.Bacc("TRN2", target_bir_lowering=False, debug=False, num_devices=8)

    xt_d = nc.dram_tensor("xt", [D, S], BF16, kind="ExternalInput")
    wqt_d = nc.dram_tensor("wqt", [D, QF], BF16, kind="ExternalInput")
    wkt_d = nc.dram_tensor("wkt", [D, KF], BF16, kind="ExternalInput")
    wvt_d = nc.dram_tensor("wvt", [D, KF], BF16, kind="ExternalInput")
    wot_d = nc.dram_tensor("wot", [QF, D], BF16, kind="ExternalInput")
    sc_d = nc.dram_tensor("sc", [1, 2], F32, kind="ExternalInput")
    yt_d = nc.dram_tensor("yt", [D, S], BF16, kind="ExternalOutput")

    with tile.TileContext(nc) as tc:
        with (
            tc.tile_pool(name="persist", bufs=1) as pers,
            tc.tile_pool(name="stg", bufs=2) as stg,
            tc.tile_pool(name="work", bufs=2) as work,
            tc.tile_pool(name="exps_p", bufs=4) as exps_p,
            tc.tile_pool(name="ysb_p", bufs=3) as ysb_p,
            tc.tile_pool(name="mm", bufs=2, space="PSUM") as mm,
            tc.tile_pool(name="scp", bufs=2, space="PSUM") as scp,
            tc.tile_pool(name="pop", bufs=2, space="PSUM") as pop,
        ):
            # ---- constants ----
            sscore_bc = pers.tile([128, 1], F32, tag="sscore")
            sout_bc = pers.tile([128, 1], F32, tag="sout")
            nc.sync.dma_start(out=sscore_bc, in_=sc_d[0:1, 0:1].to_broadcast([128, 1]))
            nc.sync.dma_start(out=sout_bc, in_=sc_d[0:1, 1:2].to_broadcast([128, 1]))
            ident = pers.tile([128, 128], BF16, tag="ident")
            make_identity(nc, ident)
            # ones row at partition 64 for the rowsum-broadcast matmul
            ones64 = pers.tile([HD + 1, HD], F32, tag="ones64")
            nc.gpsimd.memset(ones64, 1.0)

            # ---- load + sign-quantize weights (device-side sign -> bf16) ----
            def load_sign(dram, cols, tile_range, tagbase):
                # stage/sign in <=512-column chunks to keep staging slots small
                cw = min(cols, 512)
                out_tiles = []
                for t in tile_range:
                    wsb = pers.tile([128, cols], BF16, tag=f"{tagbase}{t}", name=f"{tagbase}{t}")
                    for c0 in range(0, cols, cw):
                        wstg = stg.tile([128, cw], BF16, tag="wstg")
                        nc.sync.dma_start(
                            out=wstg, in_=dram[t * 128:(t + 1) * 128, c0:c0 + cw]
                        )
                        wtmp = stg.tile([128, cw], BF16, tag="wtmp")
                        # (w >= 0) * 2 -> {2, 0}
                        nc.vector.tensor_scalar(wtmp, wstg, 0.0, 2.0, ALU.is_ge, ALU.mult)
                        # {2,0} - 1 -> {1,-1}
                        nc.vector.tensor_scalar(wsb[:, c0:c0 + cw], wtmp, 1.0, None, ALU.subtract)
                    out_tiles.append(wsb)
                return out_tiles

            # ---- load + quantize, interleaved by contraction tile so the
            # projections (which consume tile t of x and w together) can start
            # as soon as the early tiles land, hiding the load under PE work.
            # wo is loaded last: it is first needed an entire query-block later.
            x_sb = []
            wk_sb, wv_sb, wq_sb = [], [], []
            for t in range(NDT):
                xsb = pers.tile([128, S], BF16, tag=f"x{t}", name=f"x{t}")
                nc.sync.dma_start(out=xsb, in_=xt_d[t * 128:(t + 1) * 128, :])
                x_sb.append(xsb)
                wk_sb += load_sign(wkt_d, KF, range(t, t + 1), "wk")
                wv_sb += load_sign(wvt_d, KF, range(t, t + 1), "wv")
                wq_sb += load_sign(wqt_d, QF, range(t, t + 1), "wq")

            # ---- projections (feature-major: out[feat, token]) ----
            def project(w_tiles, w_col0, out_sb, out_col0):
                # out_sb[:, qb block] = (sum_kt w[kt][:, cols].T @ x[kt][:, qb]) as bf16
                for qb in range(NQB):
                    ps = mm.tile([128, QB], F32, tag="mm")
                    for t in range(NDT):
                        nc.tensor.matmul(
                            ps,
                            w_tiles[t][:, w_col0:w_col0 + 128],
                            x_sb[t][:, qb * QB:(qb + 1) * QB],
                            start=(t == 0),
                            stop=(t == NDT - 1),
                        )
                    nc.vector.tensor_copy(out_sb[:, out_col0 + qb * QB:out_col0 + (qb + 1) * QB], ps)

            k_sb = pers.tile([128, S], BF16, tag="ksb")
            project(wk_sb, 0, k_sb, 0)

            vf_sb = pers.tile([128, S], BF16, tag="vfsb")
            project(wv_sb, 0, vf_sb, 0)

            # causal masks for the 4 diagonal key-tile offsets:
            # mask[d][p, f] = 1.0 where f >= p + 128*d else 0.0
            dmask = []
            for dmi in range(4):
                msk = pers.tile([128, QB], BF16, tag=f"dmask{dmi}", name=f"dmask{dmi}")
                nc.gpsimd.memset(msk, 1.0)
                nc.gpsimd.affine_select(
                    out=msk, in_=msk, compare_op=ALU.is_ge, fill=0.0,
                    base=-128 * dmi, pattern=[[1, QB]], channel_multiplier=-1,
                )
                dmask.append(msk)

            # token-major V with a trailing ones column:
            # vtok[t][:, kv, 0:64] = V.T * (sv*so).  Only the first 4 key tiles
            # are produced up front; the rest are dripped into the attention
            # loop as PE filler (they are not needed until later query blocks).
            vtok = [
                pers.tile([128, NKV, HD + 1], BF16, tag=f"vtok{t}", name=f"vtok{t}")
                for t in range(NKT)
            ]

            def emit_vtok(t):
                vt = vtok[t]
                pst = mm.tile([128, 128], BF16, tag="mm")
                nc.tensor.transpose(pst, vf_sb[:, t * 128:(t + 1) * 128], ident)
                for kv in range(NKV):
                    nc.vector.tensor_scalar(
                        vt[:, kv, 0:HD], pst[:, kv * HD:(kv + 1) * HD],
                        sout_bc, None, ALU.mult,
                    )
                nc.vector.memset(vt[:, :, HD:HD + 1], 1.0)

            for t in range(4):
                emit_vtok(t)

            o_sb = [
                pers.tile([128, S], BF16, tag=f"osb{ft}", name=f"osb{ft}")
                for ft in range(4)
            ]
            q_sb = [
                pers.tile([128, S], BF16, tag=f"qsb{ft}", name=f"qsb{ft}")
                for ft in range(4)
            ]
            wo_sb = None  # loaded lazily after the first Q projection

            def emit_ygroup(qb, ot):
                # one partial out-projection psum group for query block qb
                q0 = qb * QB
                py = mm.tile([128, QB], F32, tag="mm")
                for it in range(4):
                    nc.tensor.matmul(
                        py,
                        wo_sb[it][:, ot * 128:(ot + 1) * 128],
                        o_sb[it][:, q0:q0 + QB],
                        start=(it == 0),
                        stop=(it == 3),
                    )
                ysb = ysb_p.tile([128, QB], BF16, tag="ysb")
                nc.vector.tensor_copy(ysb, py)
                nc.sync.dma_start(out=yt_d[ot * 128:(ot + 1) * 128, q0:q0 + QB], in_=ysb)

            # PE filler queue: small dense PE tasks (V transposes, Y-proj
            # groups for completed query blocks) dripped one per attention
            # chunk so the PE never idles while ACT computes exps (keeps the
            # HAM clock-gate warm).
            filler = [(emit_vtok, (t,)) for t in range(4, NKT)]

            def drip():
                if filler:
                    fn, args = filler.pop(0)
                    fn(*args)

            # attention: per (query block, q-tile): process the head pair
            # (ft -> rows 0:64, ft+4 -> rows 64:128) with score matmuls for the
            # two heads adjacent (they run concurrently on different PE row
            # strips) and PV lagging scores by one chunk so PE never waits on
            # the ACT exp.
            for qb in range(NQB):
                q0 = qb * QB
                nkt = 4 * (qb + 1)          # causal: key tiles 0..nkt-1
                nch = nkt // 2              # chunks of 2 key tiles
                if qb > 0:
                    filler.extend(
                        (emit_ygroup, (qb - 1, ot)) for ot in range(NDT)
                    )
                for ft in range(4):
                    if qb == 0:
                        # produce Q for this q-tile just-in-time; the next
                        # tile's projection then fills PE while this tile's
                        # attention waits on ACT.
                        project(wq_sb, ft * 128, q_sb[ft], 0)
                        if ft == 0:
                            wo_sb = load_sign(wot_d, D, range(QF // 128), "wo")
                    po_ = [
                        pop.tile([HD + 1, QB], F32, tag="po", name=f"po{qb}_{ft}_{p}")
                        for p in range(2)
                    ]

                    def emit_pv(kt, ex):
                        for p in range(2):
                            nc.tensor.matmul(
                                po_[p],
                                vtok[kt][:, p, :],
                                ex[:, p, :],
                                start=(kt == 0),
                                stop=(kt == nkt - 1),
                            )

                    prev = None
                    for kt in range(nkt):
                        # both heads' scores for one key tile in a 2-bank psum
                        # tile; bufs=2 so the next tile's scores run on PE
                        # while ACT computes this tile's exp.
                        ps = scp.tile([128, 2, QB], F32, tag="sc", bufs=2,
                                      name=f"sc{qb}_{ft}_{kt}")
                        k0 = kt * KT
                        for p in range(2):
                            r0 = p * HD
                            nc.tensor.matmul(
                                ps[:, p, :],
                                k_sb[r0:r0 + HD, k0:k0 + KT],
                                q_sb[ft][r0:r0 + HD, q0:q0 + QB],
                                start=True, stop=True,
                            )
                        ex = exps_p.tile([128, 2, QB], BF16, tag="ex", bufs=3,
                                         name=f"ex{qb}_{ft}_{kt}")
                        nc.scalar.activation(
                            out=ex[:, :, :], in_=ps[:, :, :],
                            func=ACT.Exp, scale=sscore_bc,
                        )
                        if kt >= 4 * qb:  # diagonal tile: apply causal mask
                            dmi = kt - 4 * qb
                            for p in range(2):
                                nc.vector.tensor_tensor(
                                    ex[:, p, :], ex[:, p, :], dmask[dmi], ALU.mult,
                                )
                        if prev is not None:
                            emit_pv(kt - 1, prev)
                        prev = ex
                        if kt % 2 == 0:
                            drip()
                    emit_pv(nkt - 1, prev)

                    # normalize: O[:, q] * (1 / rowsum[q]); rowsum is po row 64.
                    # Entirely off ACT (it paces the late phase): copy the
                    # rowsum row to SBUF on DVE, broadcast it across 64
                    # partitions with a K=1 ones-matmul, then take the
                    # reciprocal on DVE (approx_fast, ~4e-6 rel; inputs are
                    # well-conditioned rowsums >= 1ulp of exp(0)).
                    for p in range(2):
                        rsum = work.tile([HD + 1, QB], F32, tag="rsum")
                        nc.vector.tensor_copy(rsum[HD:HD + 1, :], po_[p][HD:HD + 1, :])
                        bcp = mm.tile([HD, QB], F32, tag="mm")
                        nc.tensor.matmul(
                            bcp,
                            ones64[HD:HD + 1, :],
                            rsum[HD:HD + 1, :],
                            start=True, stop=True,
                        )
                        bcr = work.tile([HD, QB], F32, tag="bcr")
                        nc.vector.reciprocal_approx_fast(out=bcr, in_=bcp)
                        ostg = work.tile([HD, QB], BF16, tag="ostg")
                        nc.vector.tensor_tensor(ostg, po_[p][0:HD, :], bcr, ALU.mult)
                        nc.sync.dma_start(
                            out=o_sb[ft][p * HD:(p + 1) * HD, q0:q0 + QB], in_=ostg
                        )

            # drain remaining filler (spilled Y groups) and the last block
            while filler:
                drip()
            for ot in range(NDT):
                emit_ygroup(NQB - 1, ot)

    # The ACT table-set selector assigns Exp -> exp_and_others and
    # Ln -> natural_log (first set containing each func), which thrashes the
    # table RAM (~2.7us per switch) on every ln<->exp transition in the
    # normalization chain.  Both live in natural_log_exp_and_others; steer the
    # selector there by hiding exp/ln from the other sets during this compile.
    import concourse.bacc as bacc_mod

    orig_tables = bacc_mod.get_activation_tables

    def one_set_tables(arch):
        t = orig_tables(arch)
        for name, fns in t.items():
            if name != "natural_log_exp_and_others":
                fns.discard(ACT.Exp)
                fns.discard(ACT.Ln)
        return t

    bacc_mod.get_activation_tables = one_set_tables
    try:
        nc.compile()
    finally:
        bacc_mod.get_activation_tables = orig_tables
    return nc


def _get_nc():
    global _NC
    if _NC is None:
        _NC = _build()
    return _NC


def run(inputs, trace=False, trace_cores=None):
    global _LAST_RESULTS
    x = np.asarray(inputs["x"], dtype=np.float32)
    wq = np.asarray(inputs["wq"], dtype=np.float32)
    wk = np.asarray(inputs["wk"], dtype=np.float32)
    wv = np.asarray(inputs["wv"], dtype=np.float32)
    wo = np.asarray(inputs["wo"], dtype=np.float32)

    sq = max(np.abs(wq).mean(), EPS)
    sk = max(np.abs(wk).mean(), EPS)
    sv = max(np.abs(wv).mean(), EPS)
    so = max(np.abs(wo).mean(), EPS)
    sc = np.array([[sq * sk / np.sqrt(HD), sv * so]], dtype=np.float32)

    perm_rows = np.concatenate([np.arange(h * HD, (h + 1) * HD) for h in PERM])

    in_maps = []
    for c in range(8):
        b, g = divmod(c, 4)
        wq_g = wq[QF * g:QF * (g + 1), :][perm_rows]        # [512, 2048]
        wk_g = wk[KF * g:KF * (g + 1), :]                   # [128, 2048]
        wv_g = wv[KF * g:KF * (g + 1), :]
        wo_g = wo[:, QF * g:QF * (g + 1)][:, perm_rows]     # [2048, 512]
        bf = ml_dtypes.bfloat16
        in_maps.append({
            "xt": np.ascontiguousarray(x[b].T).astype(bf),
            "wqt": np.ascontiguousarray(wq_g.T).astype(bf),
            "wkt": np.ascontiguousarray(wk_g.T).astype(bf),
            "wvt": np.ascontiguousarray(wv_g.T).astype(bf),
            "wot": np.ascontiguousarray(wo_g.T).astype(bf),
            "sc": sc,
        })

    nc = _get_nc()
    kwargs = {}
    if trace:
        kwargs["trace"] = True
        kwargs["trace_cores"] = trace_cores if trace_cores is not None else [0]
    res = bass_utils.run_bass_kernel_spmd(nc, in_maps, list(range(8)), **kwargs)
    _LAST_RESULTS = res

    y = np.empty((B, S, D), dtype=np.float32)
    for b in range(B):
        acc = np.zeros((D, S), dtype=np.float32)
        for g in range(4):
            acc += res.results[4 * b + g]["yt"].astype(np.float32)
        y[b] = acc.T
    return y


def kernel(**inputs):
    return run(inputs, trace=False)


# revision 34
# speedup vs baseline: 1.0825x; 1.0694x over previous
# BitAttention (ternary-quantized GQA transformer block) on 8 Trainium2 NeuronCores.
#
# Reference computation (see problem):
#   w_q = sign(w) * mean(|w|)            (per weight tensor, global scale)
#   q = x @ w_q(wq).T ; k = x @ w_q(wk).T ; v = x @ w_q(wv).T
#   GQA causal attention (32 q heads, 8 kv heads, head_dim 64)
#   out = attn @ w_q(wo).T
#
# Sharding (8 cores): batch (2) x kv-head-group (4).  Each core computes
# attention for 2 kv heads / 8 q heads of one batch and a partial out-proj
# over its 512 attention-output features; the host sums 4 partials per batch.
#
# Device layout: activations are kept feature-major ("transposed", [feat, token]).
# Inputs enter pre-transposed/sliced in bf16 (the kernel's compute precision);
# sign() is computed on device; the global quant scales enter as a tiny [1,2]
# fp32 tensor and are folded into the softmax exp() scale (sq*sk/sqrt(hd)) and
# the V-transpose copy (sv*so).  All matmuls are bf16 with fp32 PSUM
# accumulation; the returned partials are summed in fp32 on the host.
#
# Softmax runs without max-subtraction (scores are O(1) by construction:
# mean|w|^2-scaled dot products), so exp/rowsum/divide is exact enough in
# fp32 PSUM + bf16 storage.  Scores are computed transposed ([key, query])
# so the PV matmul needs no transposes; the rowsum rides along as a "ones"
# column appended to V; 1/rowsum is exp(-ln r) on ACT (one table set, loaded
# once).  The two heads of a pair sit on different PE row strips so their
# score matmuls execute concurrently; PV lags scores by one chunk and V-
# transposes / out-proj groups are dripped between chunks to keep the PE
# busy (HAM clock-gate warm).

import sys

for _p in ("/opt/trn_rl_repo",):
    if _p not in sys.path:
        sys.path.append(_p)

import numpy as np
import ml_dtypes

import concourse.bass as bass
import concourse.tile as tile
from concourse import bacc, mybir
from concourse import bass_utils
from concourse.masks import make_identity

F32 = mybir.dt.float32
BF16 = mybir.dt.bfloat16
ALU = mybir.AluOpType
ACT = mybir.ActivationFunctionType

D = 2048          # model dim
S = 2048          # sequence length
B = 2             # batch
HD = 64           # head dim
NQH = 8           # q heads per core
NKV = 2           # kv heads per core
QF = NQH * HD     # 512 q features per core
KF = NKV * HD     # 128 kv features per core
QB = 512          # query block (free dim of score matmuls)
KT = 128          # key tile (partition dim of transposed scores)
NKT = S // KT     # 16
NQB = S // QB     # 4
NDT = D // 128    # 16 contraction tiles
EPS = 1e-5

# processing order of local q heads: tile ft holds heads (ft, ft+4) so that
# the head's row block (64*(h//4)) matches its kv head's row block in k_sb.
PERM = [0, 4, 1, 5, 2, 6, 3, 7]

_NC = None
_LAST_RESULTS = None


def _build():
    nc = bacc.Bacc("TRN2", target_bir_lowering=False, debug=False, num_devices=8)

    xt_d = nc.dram_tensor("xt", [D, S], BF16, kind="ExternalInput")
    wqt_d = nc.dram_tensor("wqt", [D, QF], BF16, kind="ExternalInput")
    wkt_d = nc.dram_tensor("wkt", [D, KF], BF16, kind="ExternalInput")
    wvt_d = nc.dram_tensor("wvt", [D, KF], BF16, kind="ExternalInput")
    wot_d = nc.dram_tensor("wot", [QF, D], BF16, kind="ExternalInput")
    sc_d = nc.dram_tensor("sc", [1, 2], F32, kind="ExternalInput")
    yt_d = nc.dram_tensor("yt", [D, S], BF16, kind="ExternalOutput")

    with tile.TileContext(nc) as tc:
        with (
            tc.tile_pool(name="persist", bufs=1) as pers,
            tc.tile_pool(name="stg", bufs=2) as stg,
            tc.tile_pool(name="work", bufs=3) as work,
            tc.tile_pool(name="exps_p", bufs=5) as exps_p,
            tc.tile_pool(name="ysb_p", bufs=4) as ysb_p,
            tc.tile_pool(name="mm", bufs=2, space="PSUM") as mm,
            tc.tile_pool(name="scp", bufs=2, space="PSUM") as scp,
            tc.tile_pool(name="pop", bufs=2, space="PSUM") as pop,
        ):
            # ---- constants ----
            sscore_bc = pers.tile([128, 1], F32, tag="sscore")
            sout_bc = pers.tile([128, 1], F32, tag="sout")
            nc.sync.dma_start(out=sscore_bc, in_=sc_d[0:1, 0:1].to_broadcast([128, 1]))
            nc.sync.dma_start(out=sout_bc, in_=sc_d[0:1, 1:2].to_broadcast([128, 1]))
            ident = pers.tile([128, 128], BF16, tag="ident")
            make_identity(nc, ident)
            # ones row at partition 64 for the rowsum-broadcast matmul
            ones64 = pers.tile([HD + 1, HD], F32, tag="ones64")
            nc.gpsimd.memset(ones64, 1.0)

            # ---- load + sign-quantize weights (device-side sign -> bf16) ----
            def load_sign(dram, cols, tile_range, tagbase):
                # stage/sign in <=512-column chunks to keep staging slots small
                cw = min(cols, 512)
                out_tiles = []
                for t in tile_range:
                    wsb = pers.tile([128, cols], BF16, tag=f"{tagbase}{t}", name=f"{tagbase}{t}")
                    for c0 in range(0, cols, cw):
                        wstg = stg.tile([128, cw], BF16, tag="wstg")
                        nc.sync.dma_start(
                            out=wstg, in_=dram[t * 128:(t + 1) * 128, c0:c0 + cw]
                        )
                        wtmp = stg.tile([128, cw], BF16, tag="wtmp")
                        # (w >= 0) * 2 -> {2, 0}
                        nc.vector.tensor_scalar(wtmp, wstg, 0.0, 2.0, ALU.is_ge, ALU.mult)
                        # {2,0} - 1 -> {1,-1}
                        nc.vector.tensor_scalar(wsb[:, c0:c0 + cw], wtmp, 1.0, None, ALU.subtract)
                    out_tiles.append(wsb)
                return out_tiles

            # ---- load + quantize, interleaved by contraction tile so the
            # projections (which consume tile t of x and w together) can start
            # as soon as the early tiles land, hiding the load under PE work.
            # wo is loaded last: it is first needed an entire query-block later.
            x_sb = []
            wk_sb, wv_sb, wq_sb = [], [], []
            for t in range(NDT):
                xsb = pers.tile([128, S], BF16, tag=f"x{t}", name=f"x{t}")
                nc.sync.dma_start(out=xsb, in_=xt_d[t * 128:(t + 1) * 128, :])
                x_sb.append(xsb)
                wk_sb += load_sign(wkt_d, KF, range(t, t + 1), "wk")
                wv_sb += load_sign(wvt_d, KF, range(t, t + 1), "wv")
            for t in range(NDT):
                wq_sb += load_sign(wqt_d, QF, range(t, t + 1), "wq")

            # ---- projections (feature-major: out[feat, token]) ----
            def project(w_tiles, w_col0, out_sb, out_col0):
                # out_sb[:, qb block] = (sum_kt w[kt][:, cols].T @ x[kt][:, qb]) as bf16
                for qb in range(NQB):
                    ps = mm.tile([128, QB], F32, tag="mm")
                    for t in range(NDT):
                        nc.tensor.matmul(
                            ps,
                            w_tiles[t][:, w_col0:w_col0 + 128],
                            x_sb[t][:, qb * QB:(qb + 1) * QB],
                            start=(t == 0),
                            stop=(t == NDT - 1),
                        )
                    nc.vector.tensor_copy(out_sb[:, out_col0 + qb * QB:out_col0 + (qb + 1) * QB], ps)

            k_sb = pers.tile([128, S], BF16, tag="ksb")
            project(wk_sb, 0, k_sb, 0)

            vf_sb = pers.tile([128, S], BF16, tag="vfsb")
            project(wv_sb, 0, vf_sb, 0)

            # causal masks for the 4 diagonal key-tile offsets:
            # mask[d][p, f] = 1.0 where f >= p + 128*d else 0.0
            dmask = []
            for dmi in range(4):
                msk = pers.tile([128, QB], BF16, tag=f"dmask{dmi}", name=f"dmask{dmi}")
                nc.gpsimd.memset(msk, 1.0)
                nc.gpsimd.affine_select(
                    out=msk, in_=msk, compare_op=ALU.is_ge, fill=0.0,
                    base=-128 * dmi, pattern=[[1, QB]], channel_multiplier=-1,
                )
                dmask.append(msk)

            # token-major V with a trailing ones column:
            # vtok[t][:, kv, 0:64] = V.T * (sv*so).  Only the first 4 key tiles
            # are produced up front; the rest are dripped into the attention
            # loop as PE filler (they are not needed until later query blocks).
            vtok = [
                pers.tile([128, NKV, HD + 1], BF16, tag=f"vtok{t}", name=f"vtok{t}")
                for t in range(NKT)
            ]

            def emit_vtok(t):
                vt = vtok[t]
                pst = mm.tile([128, 128], BF16, tag="mm")
                nc.tensor.transpose(pst, vf_sb[:, t * 128:(t + 1) * 128], ident)
                for kv in range(NKV):
                    nc.vector.tensor_scalar(
                        vt[:, kv, 0:HD], pst[:, kv * HD:(kv + 1) * HD],
                        sout_bc, None, ALU.mult,
                    )
                nc.vector.memset(vt[:, :, HD:HD + 1], 1.0)

            for t in range(4):
                emit_vtok(t)

            o_sb = [
                pers.tile([128, S], BF16, tag=f"osb{ft}", name=f"osb{ft}")
                for ft in range(4)
            ]
            q_sb = [
                pers.tile([128, S], BF16, tag=f"qsb{ft}", name=f"qsb{ft}")
                for ft in range(4)
            ]
            wo_sb = None  # loaded lazily after the first Q projection

            def emit_ygroup(qb, ot):
                # one partial out-projection psum group for query block qb
                q0 = qb * QB
                py = mm.tile([128, QB], F32, tag="mm")
                for it in range(4):
                    nc.tensor.matmul(
                        py,
                        wo_sb[it][:, ot * 128:(ot + 1) * 128],
                        o_sb[it][:, q0:q0 + QB],
                        start=(it == 0),
                        stop=(it == 3),
                    )
                ysb = ysb_p.tile([128, QB], BF16, tag="ysb")
                nc.vector.tensor_copy(ysb, py)
                nc.sync.dma_start(out=yt_d[ot * 128:(ot + 1) * 128, q0:q0 + QB], in_=ysb)

            # PE filler queue: small dense PE tasks (V transposes, Y-proj
            # groups for completed query blocks) dripped one per attention
            # chunk so the PE never idles while ACT computes exps (keeps the
            # HAM clock-gate warm).
            filler = [(emit_vtok, (t,)) for t in range(4, NKT)]

            def drip():
                if filler:
                    fn, args = filler.pop(0)
                    fn(*args)

            # attention: per (query block, q-tile): process the head pair
            # (ft -> rows 0:64, ft+4 -> rows 64:128) with score matmuls for the
            # two heads adjacent (they run concurrently on different PE row
            # strips) and PV lagging scores by one chunk so PE never waits on
            # the ACT exp.
            for qb in range(NQB):
                q0 = qb * QB
                nkt = 4 * (qb + 1)          # causal: key tiles 0..nkt-1
                nch = nkt // 2              # chunks of 2 key tiles
                if qb > 0:
                    filler.extend(
                        (emit_ygroup, (qb - 1, ot)) for ot in range(NDT)
                    )
                for ft in range(4):
                    if qb == 0:
                        # produce Q for this q-tile just-in-time; the next
                        # tile's projection then fills PE while this tile's
                        # attention waits on ACT.
                        project(wq_sb, ft * 128, q_sb[ft], 0)
                        if ft == 0:
                            wo_sb = load_sign(wot_d, D, range(QF // 128), "wo")
                    po_ = [
                        pop.tile([HD + 1, QB], F32, tag="po", name=f"po{qb}_{ft}_{p}")
                        for p in range(2)
                    ]

                    def emit_pv(kt, ex):
                        for p in range(2):
                            nc.tensor.matmul(
                                po_[p],
                                vtok[kt][:, p, :],
                                ex[:, p, :],
                                start=(kt == 0),
                                stop=(kt == nkt - 1),
                            )

                    prev = None
                    for kt in range(nkt):
                        # both heads' scores for one key tile in a 2-bank psum
                        # tile; bufs=2 so the next tile's scores run on PE
                        # while ACT computes this tile's exp.
                        ps = scp.tile([128, 2, QB], F32, tag="sc", bufs=2,
                                      name=f"sc{qb}_{ft}_{kt}")
                        k0 = kt * KT
                        for p in range(2):
                            r0 = p * HD
                            nc.tensor.matmul(
                                ps[:, p, :],
                                k_sb[r0:r0 + HD, k0:k0 + KT],
                                q_sb[ft][r0:r0 + HD, q0:q0 + QB],
                                start=True, stop=True,
                            )
                        ex = exps_p.tile([128, 2, QB], BF16, tag="ex", bufs=4,
                                         name=f"ex{qb}_{ft}_{kt}")
                        nc.scalar.activation(
                            out=ex[:, :, :], in_=ps[:, :, :],
                            func=ACT.Exp, scale=sscore_bc,
                        )
                        if kt >= 4 * qb:  # diagonal tile: apply causal mask
                            dmi = kt - 4 * qb
                            for p in range(2):
                                nc.vector.tensor_tensor(
                                    ex[:, p, :], ex[:, p, :], dmask[dmi], ALU.mult,
                                )
                        if prev is not None:
                            emit_pv(kt - 1, prev)
                        prev = ex
                        if kt % 2 == 0:
                            drip()
                    emit_pv(nkt - 1, prev)

                    # normalize: O[:, q] * (1 / rowsum[q]); rowsum is po row 64.
                    # Entirely off ACT (it paces the late phase): copy the
                    # rowsum row to SBUF on DVE, broadcast it across 64
                    # partitions with a K=1 ones-matmul, then take the
                    # reciprocal on DVE (approx_fast, ~4e-6 rel; inputs are
                    # well-conditioned rowsums >= 1ulp of exp(0)).
                    for p in range(2):
                        rsum = work.tile([HD + 1, QB], F32, tag="rsum")
                        nc.vector.tensor_copy(rsum[HD:HD + 1, :], po_[p][HD:HD + 1, :])
                        bcp = mm.tile([HD, QB], F32, tag="mm")
                        nc.tensor.matmul(
                            bcp,
                            ones64[HD:HD + 1, :],
                            rsum[HD:HD + 1, :],
                            start=True, stop=True,
                        )
                        bcr = work.tile([HD, QB], F32, tag="bcr")
                        nc.vector.reciprocal_approx_fast(out=bcr, in_=bcp)
                        ostg = work.tile([HD, QB], BF16, tag="ostg")
                        nc.vector.tensor_tensor(ostg, po_[p][0:HD, :], bcr, ALU.mult)
                        nc.sync.dma_start(
                            out=o_sb[ft][p * HD:(p + 1) * HD, q0:q0 + QB], in_=ostg
                        )

            # drain remaining filler (spilled Y groups) and the last block
            while filler:
                drip()
            for ot in range(NDT):
                emit_ygroup(NQB - 1, ot)

    # The ACT table-set selector assigns Exp -> exp_and_others and
    # Ln -> natural_log (first set containing each func), which thrashes the
    # table RAM (~2.7us per switch) on every ln<->exp transition in the
    # normalization chain.  Both live in natural_log_exp_and_others; steer the
    # selector there by hiding exp/ln from the other sets during this compile.
    import concourse.bacc as bacc_mod

    orig_tables = bacc_mod.get_activation_tables

    def one_set_tables(arch):
        t = orig_tables(arch)
        for name, fns in t.items():
            if name != "natural_log_exp_and_others":
                fns.discard(ACT.Exp)
                fns.discard(ACT.Ln)
        return t

    bacc_mod.get_activation_tables = one_set_tables
    try:
        nc.compile()
    finally:
        bacc_mod.get_activation_tables = orig_tables
    return nc


def _get_nc():
    global _NC
    if _NC is None:
        _NC = _build()
    return _NC


def run(inputs, trace=False, trace_cores=None):
    global _LAST_RESULTS
    x = np.asarray(inputs["x"], dtype=np.float32)
    wq = np.asarray(inputs["wq"], dtype=np.float32)
    wk = np.asarray(inputs["wk"], dtype=np.float32)
    wv = np.asarray(inputs["wv"], dtype=np.float32)
    wo = np.asarray(inputs["wo"], dtype=np.float32)

    sq = max(np.abs(wq).mean(), EPS)
    sk = max(np.abs(wk).mean(), EPS)
    sv = max(np.abs(wv).mean(), EPS)
    so = max(np.abs(wo).mean(), EPS)
    sc = np.array([[sq * sk / np.sqrt(HD), sv * so]], dtype=np.float32)

    perm_rows = np.concatenate([np.arange(h * HD, (h + 1) * HD) for h in PERM])

    in_maps = []
    for c in range(8):
        b, g = divmod(c, 4)
        wq_g = wq[QF * g:QF * (g + 1), :][perm_rows]        # [512, 2048]
        wk_g = wk[KF * g:KF * (g + 1), :]                   # [128, 2048]
        wv_g = wv[KF * g:KF * (g + 1), :]
        wo_g = wo[:, QF * g:QF * (g + 1)][:, perm_rows]     # [2048, 512]
        bf = ml_dtypes.bfloat16
        in_maps.append({
            "xt": np.ascontiguousarray(x[b].T).astype(bf),
            "wqt": np.ascontiguousarray(wq_g.T).astype(bf),
            "wkt": np.ascontiguousarray(wk_g.T).astype(bf),
            "wvt": np.ascontiguousarray(wv_g.T).astype(bf),
            "wot": np.ascontiguousarray(wo_g.T).astype(bf),
            "sc": sc,
        })

    nc = _get_nc()
    kwargs = {}
    if trace:
        kwargs["trace"] = True
        kwargs["trace_cores"] = trace_cores if trace_cores is not None else [0]
    res = bass_utils.run_bass_kernel_spmd(nc, in_maps, list(range(8)), **kwargs)
    _LAST_RESULTS = res

    y = np.empty((B, S, D), dtype=np.float32)
    for b in range(B):
        acc = np.zeros((D, S), dtype=np.float32)
        for g in range(4):
            acc += res.results[4 * b + g]["yt"].astype(np.float32)
        y[b] = acc.T
    return y


def kernel(**inputs):
    return run(inputs, trace=False)


# revision 35
# speedup vs baseline: 1.1384x; 1.0517x over previous
# BitAttention (ternary-quantized GQA transformer block) on 8 Trainium2 NeuronCores.
#
# Reference computation (see problem):
#   w_q = sign(w) * mean(|w|)            (per weight tensor, global scale)
#   q = x @ w_q(wq).T ; k = x @ w_q(wk).T ; v = x @ w_q(wv).T
#   GQA causal attention (32 q heads, 8 kv heads, head_dim 64)
#   out = attn @ w_q(wo).T
#
# Sharding (8 cores): batch (2) x kv-head-group (4).  Each core computes
# attention for 2 kv heads / 8 q heads of one batch and a partial out-proj
# over its 512 attention-output features; the host sums 4 partials per batch.
#
# Device layout: activations are kept feature-major ("transposed", [feat, token]).
# Inputs enter pre-transposed/sliced in bf16 (the kernel's compute precision);
# sign() is computed on device; the global quant scales enter as a tiny [1,2]
# fp32 tensor and are folded into the softmax exp() scale (sq*sk/sqrt(hd)) and
# the V-transpose copy (sv*so).  All matmuls are bf16 with fp32 PSUM
# accumulation; the returned partials are summed in fp32 on the host.
#
# Softmax runs without max-subtraction (scores are O(1) by construction:
# mean|w|^2-scaled dot products), so exp/rowsum/divide is exact enough in
# fp32 PSUM + bf16 storage.  Scores are computed transposed ([key, query])
# so the PV matmul needs no transposes; the rowsum rides along as a "ones"
# column appended to V; 1/rowsum is exp(-ln r) on ACT (one table set, loaded
# once).  The two heads of a pair sit on different PE row strips so their
# score matmuls execute concurrently; PV lags scores by one chunk and V-
# transposes / out-proj groups are dripped between chunks to keep the PE
# busy (HAM clock-gate warm).

import sys

for _p in ("/opt/trn_rl_repo",):
    if _p not in sys.path:
        sys.path.append(_p)

import numpy as np
import ml_dtypes

import concourse.bass as bass
import concourse.tile as tile
from concourse import bacc, mybir
from concourse import bass_utils
from concourse.masks import make_identity

F32 = mybir.dt.float32
BF16 = mybir.dt.bfloat16
ALU = mybir.AluOpType
ACT = mybir.ActivationFunctionType

D = 2048          # model dim
S = 2048          # sequence length
B = 2             # batch
HD = 64           # head dim
NQH = 8           # q heads per core
NKV = 2           # kv heads per core
QF = NQH * HD     # 512 q features per core
KF = NKV * HD     # 128 kv features per core
QB = 512          # query block (free dim of score matmuls)
KT = 128          # key tile (partition dim of transposed scores)
NKT = S // KT     # 16
NQB = S // QB     # 4
NDT = D // 128    # 16 contraction tiles
EPS = 1e-5

# processing order of local q heads: tile ft holds heads (ft, ft+4) so that
# the head's row block (64*(h//4)) matches its kv head's row block in k_sb.
PERM = [0, 4, 1, 5, 2, 6, 3, 7]

_NC = None
_LAST_RESULTS = None


def _build():
    nc = bacc.Bacc("TRN2", target_bir_lowering=False, debug=False, num_devices=8)

    xt_d = nc.dram_tensor("xt", [D, S], BF16, kind="ExternalInput")
    wqt_d = nc.dram_tensor("wqt", [D, QF], BF16, kind="ExternalInput")
    wkt_d = nc.dram_tensor("wkt", [D, KF], BF16, kind="ExternalInput")
    wvt_d = nc.dram_tensor("wvt", [D, KF], BF16, kind="ExternalInput")
    wot_d = nc.dram_tensor("wot", [QF, D], BF16, kind="ExternalInput")
    sc_d = nc.dram_tensor("sc", [1, 2], F32, kind="ExternalInput")
    yt_d = nc.dram_tensor("yt", [D, S], BF16, kind="ExternalOutput")

    with tile.TileContext(nc) as tc:
        with (
            tc.tile_pool(name="persist", bufs=1) as pers,
            tc.tile_pool(name="stg", bufs=3) as stg,
            tc.tile_pool(name="work", bufs=3) as work,
            tc.tile_pool(name="exps_p", bufs=5) as exps_p,
            tc.tile_pool(name="ysb_p", bufs=4) as ysb_p,
            tc.tile_pool(name="mm", bufs=2, space="PSUM") as mm,
            tc.tile_pool(name="scp", bufs=2, space="PSUM") as scp,
            tc.tile_pool(name="pop", bufs=2, space="PSUM") as pop,
        ):
            # ---- constants ----
            sscore_bc = pers.tile([128, 1], F32, tag="sscore")
            sout_bc = pers.tile([128, 1], F32, tag="sout")
            nc.sync.dma_start(out=sscore_bc, in_=sc_d[0:1, 0:1].to_broadcast([128, 1]))
            nc.sync.dma_start(out=sout_bc, in_=sc_d[0:1, 1:2].to_broadcast([128, 1]))
            ident = pers.tile([128, 128], BF16, tag="ident")
            make_identity(nc, ident)
            # ones row at partition 64 for the rowsum-broadcast matmul
            ones64 = pers.tile([HD + 1, HD], F32, tag="ones64")
            nc.gpsimd.memset(ones64, 1.0)

            # ---- load + sign-quantize weights (device-side sign -> bf16) ----
            def load_sign(dram, cols, tile_range, tagbase):
                # stage/sign in <=512-column chunks to keep staging slots small
                cw = min(cols, 512)
                out_tiles = []
                for t in tile_range:
                    wsb = pers.tile([128, cols], BF16, tag=f"{tagbase}{t}", name=f"{tagbase}{t}")
                    for c0 in range(0, cols, cw):
                        wstg = stg.tile([128, cw], BF16, tag="wstg")
                        nc.sync.dma_start(
                            out=wstg, in_=dram[t * 128:(t + 1) * 128, c0:c0 + cw]
                        )
                        wtmp = stg.tile([128, cw], BF16, tag="wtmp")
                        # (w >= 0) * 2 -> {2, 0}
                        nc.vector.tensor_scalar(wtmp, wstg, 0.0, 2.0, ALU.is_ge, ALU.mult)
                        # {2,0} - 1 -> {1,-1}
                        nc.vector.tensor_scalar(wsb[:, c0:c0 + cw], wtmp, 1.0, None, ALU.subtract)
                    out_tiles.append(wsb)
                return out_tiles

            # ---- load + quantize, interleaved by contraction tile so the
            # projections (which consume tile t of x and w together) can start
            # as soon as the early tiles land, hiding the load under PE work.
            # wo is loaded last: it is first needed an entire query-block later.
            x_sb = []
            wk_sb, wv_sb, wq_sb = [], [], []
            for t in range(NDT):
                xsb = pers.tile([128, S], BF16, tag=f"x{t}", name=f"x{t}")
                nc.sync.dma_start(out=xsb, in_=xt_d[t * 128:(t + 1) * 128, :])
                x_sb.append(xsb)
                wk_sb += load_sign(wkt_d, KF, range(t, t + 1), "wk")
                wv_sb += load_sign(wvt_d, KF, range(t, t + 1), "wv")
            for t in range(NDT):
                wq_sb += load_sign(wqt_d, QF, range(t, t + 1), "wq")

            # ---- projections (feature-major: out[feat, token]) ----
            def project(w_tiles, w_col0, out_sb, out_col0):
                # out_sb[:, qb block] = (sum_kt w[kt][:, cols].T @ x[kt][:, qb]) as bf16
                for qb in range(NQB):
                    ps = mm.tile([128, QB], F32, tag="mm")
                    for t in range(NDT):
                        nc.tensor.matmul(
                            ps,
                            w_tiles[t][:, w_col0:w_col0 + 128],
                            x_sb[t][:, qb * QB:(qb + 1) * QB],
                            start=(t == 0),
                            stop=(t == NDT - 1),
                        )
                    nc.vector.tensor_copy(out_sb[:, out_col0 + qb * QB:out_col0 + (qb + 1) * QB], ps)

            k_sb = pers.tile([128, S], BF16, tag="ksb")
            project(wk_sb, 0, k_sb, 0)

            vf_sb = pers.tile([128, S], BF16, tag="vfsb")
            project(wv_sb, 0, vf_sb, 0)

            # causal masks for the 4 diagonal key-tile offsets:
            # mask[d][p, f] = 1.0 where f >= p + 128*d else 0.0
            dmask = []
            for dmi in range(4):
                msk = pers.tile([128, QB], BF16, tag=f"dmask{dmi}", name=f"dmask{dmi}")
                nc.gpsimd.memset(msk, 1.0)
                nc.gpsimd.affine_select(
                    out=msk, in_=msk, compare_op=ALU.is_ge, fill=0.0,
                    base=-128 * dmi, pattern=[[1, QB]], channel_multiplier=-1,
                )
                dmask.append(msk)

            # token-major V with a trailing ones column:
            # vtok[t][:, kv, 0:64] = V.T * (sv*so).  Only the first 4 key tiles
            # are produced up front; the rest are dripped into the attention
            # loop as PE filler (they are not needed until later query blocks).
            vtok = [
                pers.tile([128, NKV, HD + 1], BF16, tag=f"vtok{t}", name=f"vtok{t}")
                for t in range(NKT)
            ]

            def emit_vtok(t):
                vt = vtok[t]
                pst = mm.tile([128, 128], BF16, tag="mm")
                nc.tensor.transpose(pst, vf_sb[:, t * 128:(t + 1) * 128], ident)
                for kv in range(NKV):
                    nc.vector.tensor_scalar(
                        vt[:, kv, 0:HD], pst[:, kv * HD:(kv + 1) * HD],
                        sout_bc, None, ALU.mult,
                    )
                nc.vector.memset(vt[:, :, HD:HD + 1], 1.0)

            for t in range(4):
                emit_vtok(t)

            o_sb = [
                pers.tile([128, S], BF16, tag=f"osb{ft}", name=f"osb{ft}")
                for ft in range(4)
            ]
            q_sb = [
                pers.tile([128, S], BF16, tag=f"qsb{ft}", name=f"qsb{ft}")
                for ft in range(4)
            ]
            wo_sb = None  # loaded lazily after the first Q projection

            def emit_ygroup(qb, ot):
                # one partial out-projection psum group for query block qb
                q0 = qb * QB
                py = mm.tile([128, QB], F32, tag="mm")
                for it in range(4):
                    nc.tensor.matmul(
                        py,
                        wo_sb[it][:, ot * 128:(ot + 1) * 128],
                        o_sb[it][:, q0:q0 + QB],
                        start=(it == 0),
                        stop=(it == 3),
                    )
                ysb = ysb_p.tile([128, QB], BF16, tag="ysb")
                nc.vector.tensor_copy(ysb, py)
                nc.sync.dma_start(out=yt_d[ot * 128:(ot + 1) * 128, q0:q0 + QB], in_=ysb)

            # PE filler queue: small dense PE tasks (V transposes, Y-proj
            # groups for completed query blocks) dripped one per attention
            # chunk so the PE never idles while ACT computes exps (keeps the
            # HAM clock-gate warm).
            filler = [(emit_vtok, (t,)) for t in range(4, NKT)]

            def drip():
                if filler:
                    fn, args = filler.pop(0)
                    fn(*args)

            # attention: per (query block, q-tile): process the head pair
            # (ft -> rows 0:64, ft+4 -> rows 64:128) with score matmuls for the
            # two heads adjacent (they run concurrently on different PE row
            # strips) and PV lagging scores by one chunk so PE never waits on
            # the ACT exp.
            for qb in range(NQB):
                q0 = qb * QB
                nkt = 4 * (qb + 1)          # causal: key tiles 0..nkt-1
                nch = nkt // 2              # chunks of 2 key tiles
                if qb > 0:
                    filler.extend(
                        (emit_ygroup, (qb - 1, ot)) for ot in range(NDT)
                    )
                for ft in range(4):
                    if qb == 0:
                        # produce Q for this q-tile just-in-time; the next
                        # tile's projection then fills PE while this tile's
                        # attention waits on ACT.
                        project(wq_sb, ft * 128, q_sb[ft], 0)
                        if ft == 0:
                            wo_sb = load_sign(wot_d, D, range(QF // 128), "wo")
                    po_ = [
                        pop.tile([HD + 1, QB], F32, tag="po", name=f"po{qb}_{ft}_{p}")
                        for p in range(2)
                    ]

                    def emit_pv(kt, ex):
                        for p in range(2):
                            nc.tensor.matmul(
                                po_[p],
                                vtok[kt][:, p, :],
                                ex[:, p, :],
                                start=(kt == 0),
                                stop=(kt == nkt - 1),
                            )

                    prev = None
                    for kt in range(nkt):
                        # both heads' scores for one key tile in a 2-bank psum
                        # tile; bufs=2 so the next tile's scores run on PE
                        # while ACT computes this tile's exp.
                        ps = scp.tile([128, 2, QB], F32, tag="sc", bufs=2,
                                      name=f"sc{qb}_{ft}_{kt}")
                        k0 = kt * KT
                        for p in range(2):
                            r0 = p * HD
                            nc.tensor.matmul(
                                ps[:, p, :],
                                k_sb[r0:r0 + HD, k0:k0 + KT],
                                q_sb[ft][r0:r0 + HD, q0:q0 + QB],
                                start=True, stop=True,
                            )
                        ex = exps_p.tile([128, 2, QB], BF16, tag="ex", bufs=4,
                                         name=f"ex{qb}_{ft}_{kt}")
                        nc.scalar.activation(
                            out=ex[:, :, :], in_=ps[:, :, :],
                            func=ACT.Exp, scale=sscore_bc,
                        )
                        if kt >= 4 * qb:  # diagonal tile: apply causal mask
                            dmi = kt - 4 * qb
                            for p in range(2):
                                nc.vector.tensor_tensor(
                                    ex[:, p, :], ex[:, p, :], dmask[dmi], ALU.mult,
                                )
                        if prev is not None:
                            emit_pv(kt - 1, prev)
                        prev = ex
                        if kt % 2 == 0:
                            drip()
                    emit_pv(nkt - 1, prev)

                    # normalize: O[:, q] * (1 / rowsum[q]); rowsum is po row 64.
                    # Entirely off ACT (it paces the late phase): copy the
                    # rowsum row to SBUF on DVE, broadcast it across 64
                    # partitions with a K=1 ones-matmul, then take the
                    # reciprocal on DVE (approx_fast, ~4e-6 rel; inputs are
                    # well-conditioned rowsums >= 1ulp of exp(0)).
                    for p in range(2):
                        rsum = work.tile([HD + 1, QB], F32, tag="rsum")
                        nc.vector.tensor_copy(rsum[HD:HD + 1, :], po_[p][HD:HD + 1, :])
                        bcp = mm.tile([HD, QB], F32, tag="mm")
                        nc.tensor.matmul(
                            bcp,
                            ones64[HD:HD + 1, :],
                            rsum[HD:HD + 1, :],
                            start=True, stop=True,
                        )
                        bcr = work.tile([HD, QB], F32, tag="bcr")
                        nc.vector.reciprocal_approx_fast(out=bcr, in_=bcp)
                        ostg = work.tile([HD, QB], BF16, tag="ostg")
                        nc.vector.tensor_tensor(ostg, po_[p][0:HD, :], bcr, ALU.mult)
                        nc.sync.dma_start(
                            out=o_sb[ft][p * HD:(p + 1) * HD, q0:q0 + QB], in_=ostg
                        )

            # drain remaining filler (spilled Y groups) and the last block
            while filler:
                drip()
            for ot in range(NDT):
                emit_ygroup(NQB - 1, ot)

    # The ACT table-set selector assigns Exp -> exp_and_others and
    # Ln -> natural_log (first set containing each func), which thrashes the
    # table RAM (~2.7us per switch) on every ln<->exp transition in the
    # normalization chain.  Both live in natural_log_exp_and_others; steer the
    # selector there by hiding exp/ln from the other sets during this compile.
    import concourse.bacc as bacc_mod

    orig_tables = bacc_mod.get_activation_tables

    def one_set_tables(arch):
        t = orig_tables(arch)
        for name, fns in t.items():
            if name != "natural_log_exp_and_others":
                fns.discard(ACT.Exp)
                fns.discard(ACT.Ln)
        return t

    bacc_mod.get_activation_tables = one_set_tables
    try:
        nc.compile()
    finally:
        bacc_mod.get_activation_tables = orig_tables
    return nc


def _get_nc():
    global _NC
    if _NC is None:
        _NC = _build()
    return _NC


def run(inputs, trace=False, trace_cores=None):
    global _LAST_RESULTS
    x = np.asarray(inputs["x"], dtype=np.float32)
    wq = np.asarray(inputs["wq"], dtype=np.float32)
    wk = np.asarray(inputs["wk"], dtype=np.float32)
    wv = np.asarray(inputs["wv"], dtype=np.float32)
    wo = np.asarray(inputs["wo"], dtype=np.float32)

    sq = max(np.abs(wq).mean(), EPS)
    sk = max(np.abs(wk).mean(), EPS)
    sv = max(np.abs(wv).mean(), EPS)
    so = max(np.abs(wo).mean(), EPS)
    sc = np.array([[sq * sk / np.sqrt(HD), sv * so]], dtype=np.float32)

    perm_rows = np.concatenate([np.arange(h * HD, (h + 1) * HD) for h in PERM])

    in_maps = []
    for c in range(8):
        b, g = divmod(c, 4)
        wq_g = wq[QF * g:QF * (g + 1), :][perm_rows]        # [512, 2048]
        wk_g = wk[KF * g:KF * (g + 1), :]                   # [128, 2048]
        wv_g = wv[KF * g:KF * (g + 1), :]
        wo_g = wo[:, QF * g:QF * (g + 1)][:, perm_rows]     # [2048, 512]
        bf = ml_dtypes.bfloat16
        in_maps.append({
            "xt": np.ascontiguousarray(x[b].T).astype(bf),
            "wqt": np.ascontiguousarray(wq_g.T).astype(bf),
            "wkt": np.ascontiguousarray(wk_g.T).astype(bf),
            "wvt": np.ascontiguousarray(wv_g.T).astype(bf),
            "wot": np.ascontiguousarray(wo_g.T).astype(bf),
            "sc": sc,
        })

    nc = _get_nc()
    kwargs = {}
    if trace:
        kwargs["trace"] = True
        kwargs["trace_cores"] = trace_cores if trace_cores is not None else [0]
    res = bass_utils.run_bass_kernel_spmd(nc, in_maps, list(range(8)), **kwargs)
    _LAST_RESULTS = res

    y = np.empty((B, S, D), dtype=np.float32)
    for b in range(B):
        acc = np.zeros((D, S), dtype=np.float32)
        for g in range(4):
            acc += res.results[4 * b + g]["yt"].astype(np.float32)
        y[b] = acc.T
    return y


def kernel(**inputs):
    return run(inputs, trace=False)
